# revision 30
# baseline (speedup 1.0000x reference)
"""Trainium2 Bass kernel for nn_AbstractAttention (B=2, S=2048, D=1024, H=16, dh=64).

Sharding: 8 cores = 2 batch groups x 4 cores. Core i handles batch i//4 and
heads 4*(i%4)..+4 for QKV projection + causal attention; z^T is AllGathered
(fp16) within each 4-core group and every core then runs the output projection
for its own 256-column slice of d_model (host slices W_O per core); the host
concatenates the 4 column slices per batch.

Structure (v4):
  - host pre-transposes x to [D, S] / W to [D, he]: contiguous DMA loads, one
    2 MB DMA per input half; xk/xv on the sync HWDGE queue, xq on the scalar
    HWDGE queue so the two streams share HBM concurrently.
  - K/V/Q projections are interleaved per query-chunk into head 0's attention
    stream, so projection PE work hides under exp (ACT) time.
  - v_aug carries 64 ones-columns so the PV matmul broadcasts the softmax
    denominator into partitions 64:128 of zps for free; normalization is a
    fast-inverse bit trick + 1 Newton step (no iterative-divide RECIPROCAL).
  - exp runs on 1024-wide paired tiles (full pairs + paired diagonal blocks);
    triangular masks and one Newton op run on GpSimd to unload the DVE.
  - output projection: pass 1 (even he-chunks, heads 0/1) interleaved into
    heads 2/3; head 3's z is AllGathered per query-quarter so pass 2 (odd
    chunks) trails each quarter, leaving only ~1 quarter in the tail.
"""
import os, sys, types

sys.path.insert(0, "/opt/trn_rl_repo")
import numpy as np
import ml_dtypes

import concourse.bass as bass
import concourse.bacc as bacc
import concourse.tile as tile
from concourse import mybir
from concourse.bass_utils import run_bass_kernel_spmd

B, S, D, H, DH = 2, 2048, 1024, 16, 64
N_CORES = 8
HPC = 4            # heads per core
QC = 512           # query chunk width for score tiles
NQC = S // QC      # 4
KB = 128           # key block
NKB = S // KB      # 16
NDMC = D // 128    # 8 d_model chunks
DO = D // 4        # out-projection d_model columns per core
LEADU = 2          # score units in flight ahead of PV
MAGIC = 0x7EF311C3  # fast-inverse-reciprocal seed constant
F16 = mybir.dt.bfloat16
F32 = mybir.dt.float32
I32 = mybir.dt.int32


def _install_ntff_hook():
    """Register the axon NTFF profiling hook missing from this image's antenv."""
    if "antenv.axon_hooks" in sys.modules:
        return
    try:
        from trn_agent_boot.trn_boot import _ntff_profile_via_ctypes

        hook = _ntff_profile_via_ctypes("/opt/axon/libaxon_pjrt.so")
        if hook is None:
            return
        import antenv  # noqa: F401

        mod = types.ModuleType("antenv.axon_hooks")
        mod.get_axon_ntff_profile_hook = lambda: hook
        sys.modules["antenv.axon_hooks"] = mod
    except Exception:
        pass


def build():
    nc = bacc.Bacc("TRN2", target_bir_lowering=False, debug=False, num_devices=N_CORES)
    xq = nc.dram_tensor("xq", [D, S], F16, kind="ExternalInput")
    xk = nc.dram_tensor("xk", [D, S], F16, kind="ExternalInput")
    xv = nc.dram_tensor("xv", [D, S], F16, kind="ExternalInput")
    wq = nc.dram_tensor("wq", [D, HPC * DH], F16, kind="ExternalInput")
    wk = nc.dram_tensor("wk", [D, HPC * DH], F16, kind="ExternalInput")
    wv = nc.dram_tensor("wv", [D, HPC * DH], F16, kind="ExternalInput")
    wo = nc.dram_tensor("wo", [H * DH, DO], F16, kind="ExternalInput")
    bq = nc.dram_tensor("bq", [HPC, DH], F32, kind="ExternalInput")
    bk = nc.dram_tensor("bk", [HPC, DH], F32, kind="ExternalInput")
    bv = nc.dram_tensor("bv", [HPC, DH], F32, kind="ExternalInput")
    bo = nc.dram_tensor("bo", [DO], F32, kind="ExternalInput")
    out = nc.dram_tensor("out", [S, DO], F32, kind="ExternalOutput")

    tri_dram = nc.inline_tensor(np.triu(np.ones((128, 128), ml_dtypes.bfloat16)), name="tri_c")
    salt = int(os.environ.get("KERNEL_SALT", "0"))
    salt_dram = (
        nc.inline_tensor(np.full((1, 16 * salt), 1.0, np.float32), name="salt_c")
        if salt
        else None
    )

    with tile.TileContext(nc) as tc:
        with (
            tc.tile_pool(name="consts", bufs=1) as consts,
            tc.tile_pool(name="persist", bufs=1) as persist,
            tc.tile_pool(name="xpool", bufs=2) as xpool,
            tc.tile_pool(name="ptp", bufs=3) as ptp,
            tc.tile_pool(name="recp", bufs=2) as recp,
            tc.tile_pool(name="obp", bufs=2) as obp,
            tc.tile_pool(name="psB", bufs=3, space="PSUM") as psB,
            tc.tile_pool(name="psC", bufs=2, space="PSUM") as psC,
            tc.tile_pool(name="dram", bufs=1, space="DRAM") as dram,
        ):
            # ---- constants / weights / biases (gpsimd queue; weights first) --
            wk_sb = consts.tile([128, NDMC, HPC * DH], F16, tag="wk")
            nc.gpsimd.dma_start(out=wk_sb, in_=wk.ap().rearrange("(c p) e -> p c e", p=128))
            wv_sb = consts.tile([128, NDMC, HPC * DH], F16, tag="wv")
            nc.gpsimd.dma_start(out=wv_sb, in_=wv.ap().rearrange("(c p) e -> p c e", p=128))
            wq_sb = consts.tile([128, NDMC, HPC * DH], F16, tag="wq")
            nc.gpsimd.dma_start(out=wq_sb, in_=wq.ap().rearrange("(c p) e -> p c e", p=128))
            tri = consts.tile([128, 128], F16, tag="tri")
            nc.gpsimd.dma_start(out=tri, in_=tri_dram.ap())
            bk_sb = consts.tile([128, 2], F32, tag="bk")
            bq_sb = consts.tile([128, 2], F32, tag="bq")
            for hp in range(2):
                nc.gpsimd.dma_start(
                    out=bk_sb[:, hp : hp + 1],
                    in_=bass.AP(tensor=bk.ap().tensor, offset=128 * hp, ap=[[1, 128], [1, 1]]),
                )
                nc.gpsimd.dma_start(
                    out=bq_sb[:, hp : hp + 1],
                    in_=bass.AP(tensor=bq.ap().tensor, offset=128 * hp, ap=[[1, 128], [1, 1]]),
                )
            bv_sb = consts.tile([128, HPC, DH], F32, tag="bv")
            nc.gpsimd.dma_start(
                out=bv_sb,
                in_=bass.AP(tensor=bv.ap().tensor, offset=0, ap=[[0, 128], [64, HPC], [1, DH]]),
            )
            wo_sb = consts.tile([128, H * DH // 128, DO], F16, tag="wo")
            nc.gpsimd.dma_start(out=wo_sb, in_=wo.ap().rearrange("(c p) d -> p c d", p=128))
            bo_sb = consts.tile([128, DO], F32, tag="bo")
            nc.gpsimd.dma_start(
                out=bo_sb,
                in_=bass.AP(tensor=bo.ap().tensor, offset=0, ap=[[0, 128], [1, DO]]),
            )
            magic_sb = consts.tile([64, QC], I32, tag="magic")
            nc.vector.memset(magic_sb, MAGIC)
            if salt_dram is not None:
                salt_sb = consts.tile([1, 16 * salt], F32, tag="salt")
                nc.gpsimd.dma_start(out=salt_sb, in_=salt_dram.ap())
            # tiny dummy AllGather: absorbs the one-time ncfw/collective init
            # (~50 us) concurrently with the projection phase
            warm_in = dram.tile([64, 16], F16, tag="warmin")
            warm_out = dram.tile([4, 64, 16], F16, tag="warmout")
            nc.gpsimd.dma_start(out=warm_in, in_=tri[0:64, 0:16])
            nc.gpsimd.collective_compute(
                "AllGather",
                mybir.AluOpType.bypass,
                replica_groups=[[0, 1, 2, 3], [4, 5, 6, 7]],
                ins=[warm_in.opt()],
                outs=[warm_out.opt()],
            )

            # ---- persistent activation tiles --------------------------------
            kT = persist.tile([128, 2, S], F16, tag="kT")   # [2 heads stacked, hp, pos]
            qTs = [
                persist.tile([128, 2, QC], F16, tag=f"qT{pc}", name=f"qT{pc}")
                for pc in range(NQC)
            ]
            v_aug = persist.tile([128, NKB, HPC, 2 * DH], F16, tag="vaug")
            nc.vector.memset(v_aug[:, :, :, DH : 2 * DH], 1.0)
            zTs = [
                persist.tile([64, S], F16, tag=f"zT{h}", name=f"zT{h}")
                for h in range(HPC)
            ]
            zfull_sb = persist.tile([128, 8, S], F16, tag="zfull")
            ob_stage = persist.tile([128, S // 128, DO], F32, tag="obst")

            # ---- x loads: one 1 MB DMA per query-quarter --------------------
            def load_quarter(x_dram, pc, pfx, eng):
                t = xpool.tile([128, NDMC, QC], F16, tag=f"x{pfx}", name=f"x{pfx}{pc}")
                eng.dma_start(
                    out=t,
                    in_=x_dram.ap()
                    .rearrange("(c p) s -> p c s", p=128)[
                        :, :, QC * pc : QC * (pc + 1)
                    ],
                )
                return t

            tk, tv, tq = [], [], []
            for pc in range(NQC):
                tk.append(load_quarter(xk, pc, "k", nc.sync))
                tv.append(load_quarter(xv, pc, "v", nc.sync))
                tq.append(load_quarter(xq, pc, "q", nc.scalar))

            def proj_qk(dst_pc, dst_sl, w_sb, b_sb, t):
                for hp in range(2):
                    pj_full = psB.tile([128, 1024], F32, tag="st")
                    pj = pj_full[:, 0:512]
                    for dmc in range(NDMC):
                        nc.tensor.matmul(
                            pj,
                            w_sb[:, dmc, 128 * hp : 128 * (hp + 1)],
                            t[:, dmc],
                            start=(dmc == 0),
                            stop=(dmc == NDMC - 1),
                        )
                    nc.vector.tensor_scalar_add(
                        dst_pc[:, hp, dst_sl], pj, b_sb[:, hp : hp + 1]
                    )

            def proj_v(t, pc):
                for pb4 in range(4):
                    pv_full = psB.tile([128, 1024], F32, tag="st")
                    pv = pv_full[:, 0 : HPC * DH]
                    for dmc in range(NDMC):
                        nc.tensor.matmul(
                            pv,
                            t[:, dmc, 128 * pb4 : 128 * (pb4 + 1)],
                            wv_sb[:, dmc],
                            start=(dmc == 0),
                            stop=(dmc == NDMC - 1),
                        )
                    kb = 4 * pc + pb4
                    nc.vector.tensor_add(
                        v_aug[:, kb, :, 0:DH],
                        pv.rearrange("p (h e) -> p h e", h=HPC),
                        bv_sb,
                    )

            # ---- attention ---------------------------------------------------
            z01h = [
                dram.tile([128, S // 2], F16, tag=f"z01h{i}", name=f"z01h{i}")
                for i in range(2)
            ]
            zf01h = [
                dram.tile([4, 128, S // 2], F16, tag=f"zf01h{i}", name=f"zf01h{i}")
                for i in range(2)
            ]
            z2h = [
                dram.tile([64, S // 2], F16, tag=f"z2h{i}", name=f"z2h{i}")
                for i in range(2)
            ]
            zf2h = [
                dram.tile([4, 64, S // 2], F16, tag=f"zf2h{i}", name=f"zf2h{i}")
                for i in range(2)
            ]
            z3q = [
                dram.tile([64, S // 2], F16, tag=f"z3q{i}", name=f"z3q{i}")
                for i in range(2)
            ]
            zfh3q = [
                dram.tile([4, 64, S // 2], F16, tag=f"zfh3q{i}", name=f"zfh3q{i}")
                for i in range(2)
            ]
            RG = [[0, 1, 2, 3], [4, 5, 6, 7]]

            def attention(h, qc):
                hp, m0 = h // 2, 64 * (h % 2)
                nblk = 4 * qc + 4
                # units: pairs of full blocks, then two diagonal pairs
                units = []
                for kb in range(0, 4 * qc, 2):
                    units.append([(kb, 0, 0, 512), (kb + 1, 512, 0, 512)])
                units.append([(4 * qc, 0, 0, 512), (4 * qc + 1, 512, 128, 384)])
                units.append([(4 * qc + 2, 0, 256, 256), (4 * qc + 3, 256, 384, 128)])
                zps = psC.tile([128, QC], F32, tag="zps")
                state = {}

                def emit_scores(ui):
                    unit = units[ui]
                    st = psB.tile([128, 1024], F32, tag="st")
                    for kb, co, off, w in unit:
                        nc.tensor.matmul(
                            st[:, co : co + w],
                            kT[m0 : m0 + 64, hp, 128 * kb : 128 * (kb + 1)],
                            qTs[qc][m0 : m0 + 64, hp, off:QC],
                            start=True,
                            stop=True,
                        )
                    pt = ptp.tile([128, 1024], F16, tag="pt")
                    tw = unit[-1][1] + unit[-1][3]
                    nc.scalar.activation(
                        pt[:, 0:tw],
                        st[:, 0:tw],
                        mybir.ActivationFunctionType.Exp,
                        scale=0.125,
                    )
                    for kb, co, off, w in unit:
                        if kb >= 4 * qc:  # diagonal block: triangular mask
                            nc.vector.tensor_mul(
                                pt[:, co : co + 128], pt[:, co : co + 128], tri
                            )
                    state[ui] = pt

                def emit_pvs(ui):
                    pt = state.pop(ui)
                    for kb, co, off, w in units[ui]:
                        nc.tensor.matmul(
                            zps[:, off:QC],
                            v_aug[:, kb, h],
                            pt[:, co : co + w],
                            start=(kb == 0),
                            stop=(kb == nblk - 1),
                        )

                nu = len(units)
                for ui in range(min(LEADU, nu)):
                    emit_scores(ui)
                for ui in range(nu):
                    emit_pvs(ui)
                    if ui + LEADU < nu:
                        emit_scores(ui + LEADU)

                # z = zps[0:64] / zps[64:128] via fast-inverse + 1 Newton step
                den_i = zps[64:128, :].bitcast(I32)
                x0 = recp.tile([64, QC], F32, tag="x0")
                nc.vector.scalar_tensor_tensor(
                    x0.bitcast(I32), magic_sb, 0, den_i,
                    mybir.AluOpType.bypass, mybir.AluOpType.subtract,
                )
                e = recp.tile([64, QC], F32, tag="e")
                nc.vector.tensor_mul(e, zps[64:128, :], x0)
                x1n = recp.tile([64, QC], F32, tag="x1n")
                nc.vector.scalar_tensor_tensor(
                    x1n, e, 2.0, x0,
                    mybir.AluOpType.subtract, mybir.AluOpType.mult,
                )
                nc.vector.scalar_tensor_tensor(
                    zTs[h][:, QC * qc : QC * (qc + 1)], zps[0:64, :], -1.0, x1n,
                    mybir.AluOpType.mult, mybir.AluOpType.mult,
                )

            def allgather_h01(i):
                sl = slice((S // 2) * i, (S // 2) * (i + 1))
                nc.sync.dma_start(out=z01h[i][0:64, :], in_=zTs[0][:, sl])
                nc.sync.dma_start(out=z01h[i][64:128, :], in_=zTs[1][:, sl])
                nc.gpsimd.collective_compute(
                    "AllGather",
                    mybir.AluOpType.bypass,
                    replica_groups=RG,
                    ins=[z01h[i].opt()],
                    outs=[zf01h[i].opt()],
                )
                for j in range(4):
                    nc.scalar.dma_start(out=zfull_sb[:, 2 * j, sl], in_=zf01h[i][j])

            def allgather_h2(i):
                sl = slice((S // 2) * i, (S // 2) * (i + 1))
                nc.sync.dma_start(out=z2h[i], in_=zTs[2][:, sl])
                nc.gpsimd.collective_compute(
                    "AllGather",
                    mybir.AluOpType.bypass,
                    replica_groups=RG,
                    ins=[z2h[i].opt()],
                    outs=[zf2h[i].opt()],
                )
                for j in range(4):
                    nc.scalar.dma_start(out=zfull_sb[0:64, 2 * j + 1, sl], in_=zf2h[i][j])

            def out_proj_qb(qb, chunks, first):
                po_full = psB.tile([128, 1024], F32, tag="st")
                po = po_full[:, 0:DO]
                for ci, c in enumerate(chunks):
                    nc.tensor.matmul(
                        po,
                        zfull_sb[:, c, 128 * qb : 128 * (qb + 1)],
                        wo_sb[:, c],
                        start=(ci == 0),
                        stop=(ci == len(chunks) - 1),
                    )
                if first:
                    nc.vector.tensor_add(ob_stage[:, qb], po, bo_sb)
                else:
                    ob = obp.tile([128, DO], F32, tag="ob")
                    nc.vector.scalar_tensor_tensor(
                        ob, po, 1.0, ob_stage[:, qb],
                        mybir.AluOpType.bypass, mybir.AluOpType.add,
                    )
                    nc.sync.dma_start(
                        out=out.ap()[128 * qb : 128 * (qb + 1), :], in_=ob
                    )

            # heads 0 and 1 with interleaved projections; z AGs fire per half
            for qc in range(NQC):
                proj_qk(kT, slice(QC * qc, QC * (qc + 1)), wk_sb, bk_sb, tk[qc])
                proj_qk(qTs[qc], slice(0, QC), wq_sb, bq_sb, tq[qc])
                proj_v(tv[qc], qc)
                attention(0, qc)
                attention(1, qc)
                if qc % 2 == 1:
                    allgather_h01(qc // 2)
            # head 2
            for qc in range(NQC):
                attention(2, qc)
                if qc % 2 == 1:
                    allgather_h2(qc // 2)
            # head 3: pass 1 trails by one chunk; z AllGathered in halves
            for qc in range(NQC):
                attention(3, qc)
                if qc >= 1:
                    for qb in range(4 * (qc - 1), 4 * qc):
                        out_proj_qb(qb, [0, 2, 4, 6], first=True)
                if qc % 2 == 1:  # halves complete after qc 1 and 3
                    i = qc // 2
                    sl = slice((S // 2) * i, (S // 2) * (i + 1))
                    nc.sync.dma_start(out=z3q[i], in_=zTs[3][:, sl])
                    nc.gpsimd.collective_compute(
                        "AllGather",
                        mybir.AluOpType.bypass,
                        replica_groups=RG,
                        ins=[z3q[i].opt()],
                        outs=[zfh3q[i].opt()],
                    )
                    for j in range(4):
                        nc.scalar.dma_start(
                            out=zfull_sb[64:128, 2 * j + 1, sl], in_=zfh3q[i][j]
                        )
            for qb in range(12, 16):
                out_proj_qb(qb, [0, 2, 4, 6], first=True)
            # pass 2 (odd chunks): first position-half, then second
            for qb in range(S // 128):
                out_proj_qb(qb, [1, 3, 5, 7], first=False)

    nc.finalize()
    return nc


_CACHE = {}


def kernel(**inputs):
    _install_ntff_hook()
    nc = _CACHE.get("nc")
    if nc is None:
        nc = build()
        _CACHE["nc"] = nc

    f16 = ml_dtypes.bfloat16
    xs = {k: np.asarray(inputs[k], np.float32) for k in ("query_input", "key_input", "value_input")}
    W = {k: np.asarray(inputs[k], np.float32) for k in ("W_Q", "W_K", "W_V", "W_O")}
    b = {k: np.asarray(inputs[k], np.float32) for k in ("b_Q", "b_K", "b_V", "b_O")}
    # pre-transpose activations to [D, S] so device DMAs are contiguous
    xT16 = {k: [np.ascontiguousarray(v[g].T).astype(f16) for g in range(B)] for k, v in xs.items()}
    # pre-arrange projection weights to [D, he] per head group
    wd = {}
    for k in ("W_Q", "W_K", "W_V"):
        wd[k] = [
            np.ascontiguousarray(
                W[k][4 * r : 4 * (r + 1)].transpose(1, 0, 2).reshape(D, HPC * DH)
            ).astype(f16)
            for r in range(4)
        ]
    wo_full = W["W_O"].reshape(H * DH, D)
    wo_slices = [np.ascontiguousarray(wo_full[:, DO * r : DO * (r + 1)]).astype(f16) for r in range(4)]
    bo_slices = [np.ascontiguousarray(b["b_O"][DO * r : DO * (r + 1)]) for r in range(4)]

    in_maps = []
    for i in range(N_CORES):
        g, r = i // 4, i % 4
        in_maps.append(
            {
                "xq": xT16["query_input"][g],
                "xk": xT16["key_input"][g],
                "xv": xT16["value_input"][g],
                "wq": wd["W_Q"][r],
                "wk": wd["W_K"][r],
                "wv": wd["W_V"][r],
                "wo": wo_slices[r],
                "bq": np.ascontiguousarray(b["b_Q"][4 * r : 4 * (r + 1)]),
                "bk": np.ascontiguousarray(b["b_K"][4 * r : 4 * (r + 1)]),
                "bv": np.ascontiguousarray(b["b_V"][4 * r : 4 * (r + 1)]),
                "bo": bo_slices[r],
            }
        )

    res = run_bass_kernel_spmd(nc, in_maps, core_ids=list(range(N_CORES)))
    if os.environ.get("KERNEL_PRINT_EXEC"):
        print(f"HW exec time: {res.exec_time_ns} ns")
    outs = []
    for g in range(B):
        outs.append(
            np.concatenate([res.results[4 * g + r]["out"] for r in range(4)], axis=1)
        )
    return np.stack(outs, axis=0).astype(np.float32)
